# revision 11
# baseline (speedup 1.0000x reference)
"""Causal self-attention (B=2, T=2048, C=768, H=12) on 8 TRN2 NeuronCores.

Sharding: data-parallel over B (cores 0-3 -> b=0, cores 4-7 -> b=1), tensor
parallel over heads (3 heads per core). Each core computes q/k/v projections
for its 3 heads, causal attention, and a partial output projection; the host
sums the 4 partials per batch element and adds the output/v biases.

Attention is computed transposed: S^T[tk, tq] = K Q^T so that the softmax
denominator comes out of the ones-augmented AV matmul (V | 1) as row 64 of
the [65, 512] PSUM accumulator; exp runs on the scalar engine straight out
of PSUM with 1/sqrt(d) folded into the activation scale.

The kernel is ACT(exp)-bound in attention (~52K columns of exp through a
1.2 GHz x 128-lane pipe with ~293ns per-instruction overhead), so the
structure maximizes ACT utilization:
- heads A and B share one ACTIVATE per tk-tile: their S-matmuls run
  concurrently on PE row-strips 0-63/64-127 into one [128,1024] PSUM tile
  (2 banks) and one exp covers both. With 2-deep S-tile buffering, ACT
  streams exps back-to-back while PE runs the next S and previous AV.
- head C (second phase) batches 2 adjacent tk-tiles per ACTIVATE, with its
  K=64 S-matmuls on alternating row-strips (qC replicated to partitions
  64-127) so each batch's two matmuls execute concurrently.
- diagonal tiles are exp'd in one batched ACTIVATE per offset (A|B) with
  zero-padded masks (bank-aligned PSUM starts; the pad region's exp output
  is garbage that the mask zeroes and AV never reads).
- causal masks multiply on GpSimd (SBUF-only operands), keeping DVE free
  for PSUM evacuations (qk bias add, v copy, out-proj cast).
- q/k/v projections and the out-proj of the previous chunk are injected
  between attention steps to fill the PE slack under the ACT roofline.
- out-proj K=64 (w2c) matmuls for adjacent jt chunks are strip-paired
  (w2c replicated on partitions 64-127, ot_c on both strips).
- output partials stream back in bf16 (host sums in f32).

The v bias never touches the device: softmax rows sum to 1, so its
contribution is the constant vector out_w @ qkv_b[2C:], added on the host.
"""

import numpy as np
import ml_dtypes
from contextlib import ExitStack

import concourse.bass as bass
import concourse.tile as tile
from concourse import bacc, mybir
from concourse.bass_utils import run_bass_kernel_spmd

BF16 = mybir.dt.bfloat16
F32 = mybir.dt.float32
AF = mybir.ActivationFunctionType

B, T, C, H, D = 2, 2048, 768, 12, 64
HPC = 3          # heads per core
NCORES = 8
CC = C // 128    # 6 contraction chunks
NT = T // 128    # 16 t tiles
NJ = T // 512    # 4 tq chunks
VW = D + 1       # 65: v columns + ones column
SCALE = float(D) ** -0.5

# cross-head (A|B) diagonal masks: doubled restricted segments [m_oi|m_oi],
# consumed through a [128, 2, w] access pattern alongside the two PSUM
# segments at [0:w] and [512:512+w]
ABM_W = [512, 384, 256, 128]
ABM_OFF = [0, 1024, 1792, 2304]
ABM_TOT = 2560
# head-C diagonal masks: [m0|m1] packed (tiles at [0:512],[512:896]), then
# m2 and m3 separately (tiles at [0:256] and [512:640], exp'd separately)
CM_OFF = [ABM_TOT, ABM_TOT + 896, ABM_TOT + 896 + 256]
MASKW = ABM_TOT + 896 + 256 + 128

_cache = {}


def _xblk(j, kc):
    """Column offset of (tq-chunk j, contraction chunk kc) in the packed xt."""
    return (j * CC + kc) * 512


def _build_program():
    nc = bacc.Bacc("TRN2", target_bir_lowering=False, debug=False,
                   enable_asserts=False, num_devices=NCORES)

    xt_d = nc.dram_tensor("xt_s", [128, CC * T], BF16, kind="ExternalInput").ap()
    wqk_d = nc.dram_tensor("wqk_s", [128, CC * 384], BF16, kind="ExternalInput").ap()
    wv_d = nc.dram_tensor("wv_s", [128, CC * 192], BF16, kind="ExternalInput").ap()
    bqk_d = nc.dram_tensor("bqk_s", [128, 3], F32, kind="ExternalInput").ap()
    w2ab_d = nc.dram_tensor("w2ab_s", [128, C], BF16, kind="ExternalInput").ap()
    w2c_d = nc.dram_tensor("w2c_s", [128, C], BF16, kind="ExternalInput").ap()
    masks_d = nc.dram_tensor("masks_s", [128, MASKW], BF16, kind="ExternalInput").ap()
    outp_d = nc.dram_tensor("outp", [128, CC * T], BF16, kind="ExternalOutput").ap()

    with tile.TileContext(nc) as tc, ExitStack() as ctx:
        const = ctx.enter_context(tc.tile_pool(name="const", bufs=1))
        big = ctx.enter_context(tc.tile_pool(name="big", bufs=1))
        psum = ctx.enter_context(tc.tile_pool(name="psum", bufs=2, space="PSUM"))
        psum_pr = ctx.enter_context(tc.tile_pool(name="psum_pr", bufs=2, space="PSUM"))
        psum_av = ctx.enter_context(tc.tile_pool(name="psum_av", bufs=2, space="PSUM"))
        ppool = ctx.enter_context(tc.tile_pool(name="ppool", bufs=6))
        small = ctx.enter_context(tc.tile_pool(name="small", bufs=6))

        # warm the ACT exp table while DMAs are in flight
        warm = small.tile([1, 16], F32, tag="warm")
        nc.vector.memset(warm[:], 0.0)
        warm2 = small.tile([1, 16], F32, tag="warm")
        nc.scalar.activation(warm2[:], warm[:], AF.Exp)

        # ---- load constants/inputs (order matters: earliest consumers first)
        wqk = const.tile([128, CC * 384], BF16)
        nc.sync.dma_start(wqk[:], wqk_d[:])
        bqk = const.tile([128, 3], F32)
        nc.sync.dma_start(bqk[:], bqk_d[:])
        xt = const.tile([128, CC * T], BF16)
        # xt packed tq-chunk-major: chunk j is one contiguous block
        nc.sync.dma_start(xt[:, 0:CC * 512], xt_d[:, 0:CC * 512])
        wv = const.tile([128, CC * 192], BF16)
        nc.sync.dma_start(wv[:], wv_d[:])
        masks = const.tile([128, MASKW], BF16)
        nc.sync.dma_start(masks[:], masks_d[:])

        def emit_load_xt(j):
            sl = slice(j * CC * 512, (j + 1) * CC * 512)
            nc.sync.dma_start(xt[:, sl], xt_d[:, sl])

        # ---- persistent intermediates
        qt1 = big.tile([128, T], BF16)   # qA (p 0-63) | qB (p 64-127), [d, t]
        kt1 = big.tile([128, T], BF16)   # kA | kB
        qkt2 = big.tile([128, T], BF16)  # qC | kC
        kt2 = big.tile([64, T], BF16)    # kC shifted to partitions 0-63
        qt_c2 = big.tile([128, T], BF16)  # qC replicated to partitions 64-127
        vbuf = big.tile([128, NT * HPC * VW], BF16)  # per t-chunk: [vA 1|vB 1|vC 1]
        ot_ab = big.tile([128, T], BF16)  # O.T heads A,B (out-proj rhs chunk 0)
        ot_c = big.tile([128, T], BF16)   # O.T head C on both 64-row strips

        nc.vector.memset(vbuf[:], 1.0)

        def s_operands(h, i, j, c0):
            """(lhsT=k-tile, rhs=q-chunk) for head h, tk-tile i, tq-chunk j.
            Heads A/B live on fixed row strips; head C alternates strips by
            tile parity so adjacent tiles run concurrently."""
            it = slice(i * 128, (i + 1) * 128)
            qs = slice(j * 512 + c0, (j + 1) * 512)
            if h == 0:
                return kt1[0:64, it], qt1[0:64, qs]
            if h == 1:
                return kt1[64:128, it], qt1[64:128, qs]
            if i % 2 == 0:
                return kt2[0:64, it], qkt2[0:64, qs]
            return qkt2[64:128, it], qt_c2[64:128, qs]

        def v_ap(h, i):
            off = i * HPC * VW + h * VW
            return vbuf[:, off:off + VW]

        qk_dest = [qt1, kt1, qkt2]

        def emit_qk_group(jt, j):
            ps = psum_pr.tile([128, 512], F32, tag="proj", name=f"qk_{jt}_{j}")
            for kc in range(CC):
                nc.tensor.matmul(
                    ps[:],
                    wqk[:, kc * 384 + jt * 128: kc * 384 + (jt + 1) * 128],
                    xt[:, _xblk(j, kc):_xblk(j, kc) + 512],
                    start=(kc == 0), stop=(kc == CC - 1),
                )
            nc.vector.tensor_scalar_add(
                qk_dest[jt][:, j * 512:(j + 1) * 512], ps[:], bqk[:, jt:jt + 1])

        def emit_shift(j):
            js = slice(j * 512, (j + 1) * 512)
            nc.sync.dma_start(kt2[:, js], qkt2[64:128, js])

        def emit_qshift(j):
            js = slice(j * 512, (j + 1) * 512)
            nc.sync.dma_start(qt_c2[64:128, js], qkt2[0:64, js])

        def emit_v_group(ti):
            ps = psum_pr.tile([128, 192], F32, tag="proj", name=f"v_{ti}")
            base = _xblk(ti // 4, 0) + (ti % 4) * 128
            for kc in range(CC):
                nc.tensor.matmul(
                    ps[:],
                    xt[:, base + kc * 512: base + kc * 512 + 128],
                    wv[:, kc * 192:(kc + 1) * 192],
                    start=(kc == 0), stop=(kc == CC - 1),
                )
            dst = vbuf[:, ti * HPC * VW:(ti + 1) * HPC * VW]
            dst = dst.rearrange("p (h x) -> p h x", h=HPC)[:, :, 0:D]
            nc.vector.tensor_copy(
                dst, ps[:].rearrange("p (h x) -> p h x", h=HPC))

        def emit_outproj_pair(jtp, j):
            """Out-proj for jt = 2*jtp, 2*jtp+1: the two K=64 w2c matmuls run
            concurrently on row strips 0-63 / 64-127."""
            js = slice(j * 512, (j + 1) * 512)
            ps = [psum_pr.tile([128, 512], F32, tag="proj",
                               name=f"op_{jtp}_{j}_{u}") for u in range(2)]
            for u in range(2):
                jt = 2 * jtp + u
                nc.tensor.matmul(ps[u][:], w2ab[:, jt * 128:(jt + 1) * 128],
                                 ot_ab[:, js], start=True, stop=False,
                                 skip_group_check=True)
            nc.tensor.matmul(ps[0][:], w2c[0:64, (2 * jtp) * 128:(2 * jtp + 1) * 128],
                             ot_c[0:64, js], start=False, stop=True,
                             skip_group_check=True)
            nc.tensor.matmul(ps[1][:], w2c[64:128, (2 * jtp + 1) * 128:(2 * jtp + 2) * 128],
                             ot_c[64:128, js], start=False, stop=True,
                             skip_group_check=True)
            for u in range(2):
                jt = 2 * jtp + u
                ob = small.tile([128, 512], BF16, tag="ob", name=f"ob_{jtp}_{j}_{u}")
                nc.vector.tensor_copy(ob[:], ps[u][:])
                nc.sync.dma_start(
                    outp_d[:, jt * T + j * 512: jt * T + (j + 1) * 512], ob[:])

        # warm-up matmuls: PE activity during the input DMA wait so the HAM
        # clock-gate reaches K=8/8 before real work starts
        wz = const.tile([128, 512], BF16, name="wz")
        nc.vector.memset(wz[:], 0.0)
        wps = psum.tile([128, 1024], F32, tag="s", name="wps")
        for _ in range(10):
            nc.tensor.matmul(wps[:, 0:512], wz[:, 0:128], wz[:],
                             start=True, stop=True)

        # prologue: just enough projections to start attention chunk 0
        emit_qk_group(0, 0)
        emit_qk_group(1, 0)
        emit_qk_group(2, 0)
        emit_shift(0)
        emit_qshift(0)
        emit_v_group(0)
        emit_v_group(1)

        # remaining inputs, issued behind the chunk-0 working set
        emit_load_xt(1)
        w2ab = const.tile([128, C], BF16)
        nc.sync.dma_start(w2ab[:], w2ab_d[:])
        w2c = const.tile([128, C], BF16)
        nc.sync.dma_start(w2c[:], w2c_d[:])
        emit_load_xt(2)
        emit_load_xt(3)

        for j in range(NJ):
            js = slice(j * 512, (j + 1) * 512)

            # work to inject between attention steps of this chunk
            inj = []
            if j == 0:
                inj += [lambda: emit_v_group(2), lambda: emit_v_group(3)]
            if j + 1 < NJ:
                inj += [lambda jt=jt: emit_qk_group(jt, j + 1) for jt in range(3)]
                inj.append(lambda: emit_shift(j + 1))
                inj.append(lambda: emit_qshift(j + 1))
                inj += [lambda ti=ti: emit_v_group(ti)
                        for ti in range(4 * (j + 1), 4 * (j + 1) + 4)]
            if j - 1 >= 0:
                inj += [lambda jtp=jtp: emit_outproj_pair(jtp, j - 1)
                        for jtp in range(3)]

            total_steps = 6 * j + 8
            step = 0

            def maybe_inject():
                nonlocal step
                step += 1
                remaining_slots = total_steps - step + 1
                k = -(-len(inj) // max(1, remaining_slots))  # ceil
                for _ in range(min(k, len(inj))):
                    inj.pop(0)()

            # ---- phase 1: heads A,B -- one exp per tk-tile covers both
            av = {h: psum_av.tile([VW, 512], F32, tag="av",
                                  name=f"av_{h}_{j}") for h in (0, 1)}
            started = {h: False for h in (0, 1)}
            prev = []   # (h, i, pt_ap, col0) from previous step

            def flush_av(last=False):
                for (h, i, pt_ap, c0) in prev:
                    nc.tensor.matmul(
                        av[h][:, c0:512], v_ap(h, i), pt_ap,
                        start=(not started[h]), stop=last,
                        skip_group_check=True,
                    )
                    started[h] = True
                prev.clear()

            for i in range(4 * j + 4):
                oi = i - 4 * j          # >=0 on the diagonal
                c0 = max(0, 128 * oi)
                w = 512 - c0
                sp = psum.tile([128, 1024], F32, tag="s", name=f"sp_{j}_{i}")
                for h in (0, 1):
                    lhsT, rhs = s_operands(h, i, j, c0)
                    nc.tensor.matmul(sp[:, h * 512:h * 512 + w], lhsT, rhs,
                                     start=True, stop=True,
                                     skip_group_check=True)
                pt = ppool.tile([128, 1024], BF16, tag="pt", name=f"pt_{j}_{i}")
                if oi < 0:
                    nc.scalar.activation(pt[:], sp[:], AF.Exp, scale=SCALE)
                else:
                    # one exp over both heads' written segments via a
                    # [128, 2, w] strided view (skips the [w:512] gap)
                    sp2 = sp.rearrange("p (s x) -> p s x", s=2)[:, :, 0:w]
                    pt2 = pt.rearrange("p (s x) -> p s x", s=2)[:, :, 0:w]
                    nc.scalar.activation(pt2, sp2, AF.Exp, scale=SCALE)
                    mk = masks[:, ABM_OFF[oi]:ABM_OFF[oi] + 2 * w] \
                        .rearrange("p (s x) -> p s x", s=2)
                    nc.gpsimd.tensor_mul(pt2, pt2, mk)
                flush_av()
                prev.append((0, i, pt[:, 0:w], c0))
                prev.append((1, i, pt[:, 512:512 + w], c0))
                maybe_inject()
            flush_av(last=True)

            for h in (0, 1):
                emit_normalize(nc, small, av[h], h, js, ot_ab, ot_c, j)

            # ---- phase 2: head C -- two tk-tiles per exp batch
            av_c = psum_av.tile([VW, 512], F32, tag="av", name=f"av_2_{j}")
            started_c = False
            prev_c = []

            def flush_av_c(last=False):
                nonlocal started_c
                for (i, pt_ap, c0) in prev_c:
                    nc.tensor.matmul(
                        av_c[:, c0:512], v_ap(2, i), pt_ap,
                        start=(not started_c), stop=last,
                        skip_group_check=True,
                    )
                    started_c = True
                prev_c.clear()

            for b0 in range(2 * j + 2):
                i0 = 2 * b0
                ois = [i0 - 4 * j, i0 + 1 - 4 * j]   # diagonal offsets
                c0s = [max(0, 128 * o) for o in ois]
                ws = [512 - c for c in c0s]
                sp = psum.tile([128, 1024], F32, tag="s", name=f"spc_{j}_{b0}")
                for u in range(2):
                    lhsT, rhs = s_operands(2, i0 + u, j, c0s[u])
                    nc.tensor.matmul(sp[:, u * 512:u * 512 + ws[u]], lhsT, rhs,
                                     start=True, stop=True,
                                     skip_group_check=True)
                pt = ppool.tile([128, 1024], BF16, tag="pt", name=f"ptc_{j}_{b0}")
                if ois[0] < 0 and ois[1] < 0:
                    # both fully below the diagonal: contiguous 1024 exp
                    nc.scalar.activation(pt[:], sp[:], AF.Exp, scale=SCALE)
                elif ois[0] == 0:
                    # diagonal batch 1: tiles at [0:512],[512:896] contiguous
                    nc.scalar.activation(pt[:, 0:896], sp[:, 0:896],
                                         AF.Exp, scale=SCALE)
                    nc.gpsimd.tensor_mul(pt[:, 0:896], pt[:, 0:896],
                                         masks[:, CM_OFF[0]:CM_OFF[0] + 896])
                else:
                    # diagonal batch 2: tiles at [0:256] and [512:640]
                    nc.scalar.activation(pt[:, 0:256], sp[:, 0:256],
                                         AF.Exp, scale=SCALE)
                    nc.scalar.activation(pt[:, 512:640], sp[:, 512:640],
                                         AF.Exp, scale=SCALE)
                    nc.gpsimd.tensor_mul(pt[:, 0:256], pt[:, 0:256],
                                         masks[:, CM_OFF[1]:CM_OFF[1] + 256])
                    nc.gpsimd.tensor_mul(pt[:, 512:640], pt[:, 512:640],
                                         masks[:, CM_OFF[2]:CM_OFF[2] + 128])
                flush_av_c()
                prev_c.append((i0, pt[:, 0:ws[0]], c0s[0]))
                prev_c.append((i0 + 1, pt[:, 512:512 + ws[1]], c0s[1]))
                maybe_inject()
            flush_av_c(last=True)

            emit_normalize(nc, small, av_c, 2, js, ot_ab, ot_c, j)
            if j >= 0:
                nc.sync.dma_start(ot_c[64:128, js], ot_c[0:64, js])

            # leftover injections for this chunk
            while inj:
                inj.pop(0)()

        # epilogue: out-projection for the last chunk
        for jtp in range(3):
            emit_outproj_pair(jtp, NJ - 1)

    nc.compile()
    return nc


def emit_normalize(nc, small, av, h, js, ot_ab, ot_c, j):
    """O.T[d, tq] = av[0:64] / av[64] into the out-proj rhs buffers."""
    recip = small.tile([1, 512], F32, tag="recip", name=f"recip_{h}_{j}")
    den = small.tile([1, 512], F32, tag="den", name=f"den_{h}_{j}")
    nc.vector.tensor_copy(den[:], av[D:VW, :])
    # custom-DVE ops read garbage from PSUM; SBUF source only
    nc.vector.reciprocal_approx_fast(recip[:], den[:])
    rb = small.tile([64, 512], F32, tag="rb", name=f"rb_{h}_{j}")
    nc.gpsimd.partition_broadcast(rb[:], recip[:])
    if h == 0:
        dst = ot_ab[0:64, js]
    elif h == 2:
        dst = ot_c[0:64, js]
    else:
        dst = small.tile([64, 512], BF16, tag="otb", name=f"otb_{j}")
    nc.vector.tensor_mul(dst[:], av[0:D, :], rb[:])
    if h == 1:
        nc.sync.dma_start(ot_ab[64:128, js], dst[:])


def _prep_in_maps(x, qkv_w, qkv_b, out_w):
    bf = ml_dtypes.bfloat16
    in_maps = []

    # causal mask segments (keep when f >= oi*128 + p)
    p = np.arange(128)[:, None]

    def seg(oi):
        f = np.arange(128 * oi, 512)[None, :]
        return (f >= oi * 128 + p)

    # cross-head (A|B): doubled restricted segments [m_oi | m_oi]
    ab = [np.concatenate([seg(oi), seg(oi)], axis=1) for oi in range(4)]
    # head-C: [m0 | m1] packed, then m2 and m3 separately
    cm = [np.concatenate([seg(0), seg(1)], axis=1), seg(2), seg(3)]
    masks_s = np.concatenate(ab + cm, axis=1).astype(bf)
    assert masks_s.shape[1] == MASKW, masks_s.shape

    for c in range(NCORES):
        b = c // 4
        h0 = (c % 4) * HPC
        hs = [h0, h0 + 1, h0 + 2]

        xT = np.ascontiguousarray(x[b].T.astype(np.float32))  # [768, 2048]
        # pack tq-chunk-major: [128, (j, kc, 512)]
        xt_s = xT.reshape(CC, 128, NJ, 512).transpose(1, 2, 0, 3) \
            .reshape(128, CC * T)

        qr = lambda h: qkv_w[h * D:(h + 1) * D]
        kr = lambda h: qkv_w[C + h * D: C + (h + 1) * D]
        vr = lambda h: qkv_w[2 * C + h * D: 2 * C + (h + 1) * D]
        qb = lambda h: qkv_b[h * D:(h + 1) * D]
        kb = lambda h: qkv_b[C + h * D: C + (h + 1) * D]

        wqk = np.concatenate([qr(hs[0]), qr(hs[1]), kr(hs[0]), kr(hs[1]),
                              qr(hs[2]), kr(hs[2])], axis=0)  # [384, 768]
        wqk_s = np.ascontiguousarray(wqk.T).reshape(CC, 128, 384) \
            .transpose(1, 0, 2).reshape(128, CC * 384)
        wv_ = np.concatenate([vr(h) for h in hs], axis=0)      # [192, 768]
        wv_s = np.ascontiguousarray(wv_.T).reshape(CC, 128, 192) \
            .transpose(1, 0, 2).reshape(128, CC * 192)

        bqk = np.concatenate([qb(hs[0]), qb(hs[1]), kb(hs[0]), kb(hs[1]),
                              qb(hs[2]), kb(hs[2])])
        bqk_s = np.ascontiguousarray(bqk.reshape(3, 128).T).astype(np.float32)

        ch_ab = np.r_[hs[0] * D:(hs[0] + 1) * D, hs[1] * D:(hs[1] + 1) * D]
        ch_c = np.r_[hs[2] * D:(hs[2] + 1) * D]
        w2ab_s = np.ascontiguousarray(out_w[:, ch_ab].T)  # [128, 768]
        w2c_half = np.ascontiguousarray(out_w[:, ch_c].T)  # [64, 768]
        w2c_s = np.concatenate([w2c_half, w2c_half], axis=0)  # [128, 768]

        in_maps.append({
            "xt_s": np.ascontiguousarray(xt_s).astype(bf),
            "wqk_s": np.ascontiguousarray(wqk_s).astype(bf),
            "wv_s": np.ascontiguousarray(wv_s).astype(bf),
            "bqk_s": bqk_s,
            "w2ab_s": w2ab_s.astype(bf),
            "w2c_s": w2c_s.astype(bf),
            "masks_s": masks_s,
        })
    return in_maps


def _assemble(results, qkv_b, out_w, out_b):
    out = np.zeros((B, T, C), dtype=np.float32)
    for c in range(NCORES):
        b = c // 4
        outp = np.asarray(results[c]["outp"]).astype(np.float32)  # [128, CC*T]
        outT = outp.reshape(128, CC, T).transpose(1, 0, 2).reshape(C, T)
        out[b] += outT.T
    # v-bias contribution (softmax rows sum to 1) + output bias
    const = out_w.astype(np.float32) @ qkv_b[2 * C:].astype(np.float32) \
        + out_b.astype(np.float32)
    out += const[None, None, :]
    return out


def run(x, qkv_w, qkv_b, out_w, out_b, trace=False, tmpdir=None):
    if "nc" not in _cache:
        _cache["nc"] = _build_program()
    nc = _cache["nc"]
    x = np.asarray(x, dtype=np.float32)
    qkv_w = np.asarray(qkv_w, dtype=np.float32)
    qkv_b = np.asarray(qkv_b, dtype=np.float32)
    out_w = np.asarray(out_w, dtype=np.float32)
    out_b = np.asarray(out_b, dtype=np.float32)
    in_maps = _prep_in_maps(x, qkv_w, qkv_b, out_w)
    res = run_bass_kernel_spmd(nc, in_maps, list(range(NCORES)), trace=trace,
                               tmpdir=tmpdir)
    out = _assemble(res.results, qkv_b, out_w, out_b)
    return out, res


def kernel(x, qkv_w, qkv_b, out_w, out_b):
    out, _ = run(x, qkv_w, qkv_b, out_w, out_b, trace=False)
    return out


# revision 12
# speedup vs baseline: 1.5261x; 1.5261x over previous
"""Causal self-attention (B=2, T=2048, C=768, H=12) on 8 TRN2 NeuronCores.

Sharding: data-parallel over B (cores 0-3 -> b=0, cores 4-7 -> b=1), tensor
parallel over heads (3 heads per core). Each core computes q/k/v projections
for its 3 heads, causal attention, and a partial output projection; the host
sums the 4 partials per batch element and adds the output/v biases.

Attention is computed transposed: S^T[tk, tq] = K Q^T so that the softmax
denominator comes out of the ones-augmented AV matmul (V | 1) as row 64 of
the [65, 512] PSUM accumulator; exp runs on the scalar engine straight out
of PSUM with 1/sqrt(d) folded into the activation scale.

The kernel is ACT(exp)-bound in attention (~52K columns of exp through a
1.2 GHz x 128-lane pipe with ~293ns per-instruction overhead), so the
structure maximizes ACT utilization:
- heads A and B share one ACTIVATE per tk-tile: their S-matmuls run
  concurrently on PE row-strips 0-63/64-127 into one [128,1024] PSUM tile
  (2 banks) and one exp covers both. With 2-deep S-tile buffering, ACT
  streams exps back-to-back while PE runs the next S and previous AV.
- head C (second phase) batches 2 adjacent tk-tiles per ACTIVATE, with its
  K=64 S-matmuls on alternating row-strips (qC replicated to partitions
  64-127) so each batch's two matmuls execute concurrently.
- diagonal tiles are exp'd in one batched ACTIVATE per offset (A|B) with
  zero-padded masks (bank-aligned PSUM starts; the pad region's exp output
  is garbage that the mask zeroes and AV never reads).
- causal masks multiply on GpSimd (SBUF-only operands), keeping DVE free
  for PSUM evacuations (qk bias add, v copy, out-proj cast).
- q/k/v projections and the out-proj of the previous chunk are injected
  between attention steps to fill the PE slack under the ACT roofline.
- out-proj K=64 (w2c) matmuls for adjacent jt chunks are strip-paired
  (w2c replicated on partitions 64-127, ot_c on both strips).
- output partials stream back in bf16 (host sums in f32).

The v bias never touches the device: softmax rows sum to 1, so its
contribution is the constant vector out_w @ qkv_b[2C:], added on the host.
"""

import numpy as np
import ml_dtypes
from contextlib import ExitStack

import concourse.bass as bass
import concourse.tile as tile
from concourse import bacc, mybir
from concourse.bass_utils import run_bass_kernel_spmd

BF16 = mybir.dt.bfloat16
F32 = mybir.dt.float32
AF = mybir.ActivationFunctionType

B, T, C, H, D = 2, 2048, 768, 12, 64
HPC = 3          # heads per core
NCORES = 8
CC = C // 128    # 6 contraction chunks
NT = T // 128    # 16 t tiles
NJ = T // 512    # 4 tq chunks
VW = D + 1       # 65: v columns + ones column
SCALE = float(D) ** -0.5

# cross-head (A|B) diagonal masks: doubled restricted segments [m_oi|m_oi],
# consumed through a [128, 2, w] access pattern alongside the two PSUM
# segments at [0:w] and [512:512+w]
ABM_W = [512, 384, 256, 128]
ABM_OFF = [0, 1024, 1792, 2304]
ABM_TOT = 2560
# head-C diagonal masks: [m0|m1] packed (tiles at [0:512],[512:896]), then
# m2 and m3 separately (tiles at [0:256] and [512:640], exp'd separately)
CM_OFF = [ABM_TOT, ABM_TOT + 896, ABM_TOT + 896 + 256]
MASKW = ABM_TOT + 896 + 256 + 128

_cache = {}


def _xblk(j, kc):
    """Column offset of (tq-chunk j, contraction chunk kc) in the packed xt."""
    return (j * CC + kc) * 512


def _build_program():
    nc = bacc.Bacc("TRN2", target_bir_lowering=False, debug=False,
                   enable_asserts=False, num_devices=NCORES)

    xt_d = nc.dram_tensor("xt_s", [128, CC * T], BF16, kind="ExternalInput").ap()
    wqk_d = nc.dram_tensor("wqk_s", [128, CC * 384], BF16, kind="ExternalInput").ap()
    wv_d = nc.dram_tensor("wv_s", [128, CC * 192], BF16, kind="ExternalInput").ap()
    bqk_d = nc.dram_tensor("bqk_s", [128, 3], F32, kind="ExternalInput").ap()
    w2ab_d = nc.dram_tensor("w2ab_s", [128, C], BF16, kind="ExternalInput").ap()
    w2c_d = nc.dram_tensor("w2c_s", [128, C], BF16, kind="ExternalInput").ap()
    masks_d = nc.dram_tensor("masks_s", [128, MASKW], BF16, kind="ExternalInput").ap()
    outp_d = nc.dram_tensor("outp", [128, CC * T], BF16, kind="ExternalOutput").ap()

    with tile.TileContext(nc) as tc, ExitStack() as ctx:
        const = ctx.enter_context(tc.tile_pool(name="const", bufs=1))
        big = ctx.enter_context(tc.tile_pool(name="big", bufs=1))
        psum = ctx.enter_context(tc.tile_pool(name="psum", bufs=2, space="PSUM"))
        psum_pr = ctx.enter_context(tc.tile_pool(name="psum_pr", bufs=2, space="PSUM"))
        psum_av = ctx.enter_context(tc.tile_pool(name="psum_av", bufs=2, space="PSUM"))
        ppool = ctx.enter_context(tc.tile_pool(name="ppool", bufs=6))
        small = ctx.enter_context(tc.tile_pool(name="small", bufs=6))

        # warm the ACT exp table while DMAs are in flight
        warm = small.tile([1, 16], F32, tag="warm")
        nc.vector.memset(warm[:], 0.0)
        warm2 = small.tile([1, 16], F32, tag="warm")
        nc.scalar.activation(warm2[:], warm[:], AF.Exp)

        # ---- load constants/inputs (order matters: earliest consumers first)
        wqk = const.tile([128, CC * 384], BF16)
        nc.sync.dma_start(wqk[:], wqk_d[:])
        bqk = const.tile([128, 3], F32)
        nc.sync.dma_start(bqk[:], bqk_d[:])
        xt = const.tile([128, CC * T], BF16)
        # xt packed tq-chunk-major: chunk j is one contiguous block
        nc.sync.dma_start(xt[:, 0:CC * 512], xt_d[:, 0:CC * 512])
        wv = const.tile([128, CC * 192], BF16)
        nc.sync.dma_start(wv[:], wv_d[:])
        masks = const.tile([128, MASKW], BF16)
        nc.sync.dma_start(masks[:], masks_d[:])

        def emit_load_xt(j):
            sl = slice(j * CC * 512, (j + 1) * CC * 512)
            nc.sync.dma_start(xt[:, sl], xt_d[:, sl])

        # ---- persistent intermediates
        qt1 = big.tile([128, T], BF16)   # qA (p 0-63) | qB (p 64-127), [d, t]
        kt1 = big.tile([128, T], BF16)   # kA | kB
        qkt2 = big.tile([128, T], BF16)  # qC | kC
        kt2 = big.tile([64, T], BF16)    # kC shifted to partitions 0-63
        qt_c2 = big.tile([128, T], BF16)  # qC replicated to partitions 64-127
        vbuf = big.tile([128, NT * HPC * VW], BF16)  # per t-chunk: [vA 1|vB 1|vC 1]
        ot_ab = big.tile([128, T], BF16)  # O.T heads A,B (out-proj rhs chunk 0)
        ot_c = big.tile([128, T], BF16)   # O.T head C on both 64-row strips

        nc.vector.memset(vbuf[:], 1.0)

        def s_operands(h, i, j, c0):
            """(lhsT=k-tile, rhs=q-chunk) for head h, tk-tile i, tq-chunk j.
            Heads A/B live on fixed row strips; head C alternates strips by
            tile parity so adjacent tiles run concurrently."""
            it = slice(i * 128, (i + 1) * 128)
            qs = slice(j * 512 + c0, (j + 1) * 512)
            if h == 0:
                return kt1[0:64, it], qt1[0:64, qs]
            if h == 1:
                return kt1[64:128, it], qt1[64:128, qs]
            if i % 2 == 0:
                return kt2[0:64, it], qkt2[0:64, qs]
            return qkt2[64:128, it], qt_c2[64:128, qs]

        def v_ap(h, i):
            off = i * HPC * VW + h * VW
            return vbuf[:, off:off + VW]

        qk_dest = [qt1, kt1, qkt2]

        def emit_qk_group(jt, j):
            ps = psum_pr.tile([128, 512], F32, tag="proj", name=f"qk_{jt}_{j}")
            for kc in range(CC):
                nc.tensor.matmul(
                    ps[:],
                    wqk[:, kc * 384 + jt * 128: kc * 384 + (jt + 1) * 128],
                    xt[:, _xblk(j, kc):_xblk(j, kc) + 512],
                    start=(kc == 0), stop=(kc == CC - 1),
                )
            nc.vector.tensor_scalar_add(
                qk_dest[jt][:, j * 512:(j + 1) * 512], ps[:], bqk[:, jt:jt + 1])

        def emit_shift(j):
            js = slice(j * 512, (j + 1) * 512)
            nc.sync.dma_start(kt2[:, js], qkt2[64:128, js])

        def emit_qshift(j):
            js = slice(j * 512, (j + 1) * 512)
            nc.sync.dma_start(qt_c2[64:128, js], qkt2[0:64, js])

        def emit_v_group(ti):
            ps = psum_pr.tile([128, 192], F32, tag="proj", name=f"v_{ti}")
            base = _xblk(ti // 4, 0) + (ti % 4) * 128
            for kc in range(CC):
                nc.tensor.matmul(
                    ps[:],
                    xt[:, base + kc * 512: base + kc * 512 + 128],
                    wv[:, kc * 192:(kc + 1) * 192],
                    start=(kc == 0), stop=(kc == CC - 1),
                )
            dst = vbuf[:, ti * HPC * VW:(ti + 1) * HPC * VW]
            dst = dst.rearrange("p (h x) -> p h x", h=HPC)[:, :, 0:D]
            nc.vector.tensor_copy(
                dst, ps[:].rearrange("p (h x) -> p h x", h=HPC))

        def emit_outproj_pair(jtp, j):
            """Out-proj for jt = 2*jtp, 2*jtp+1: the two K=64 w2c matmuls run
            concurrently on row strips 0-63 / 64-127."""
            js = slice(j * 512, (j + 1) * 512)
            ps = [psum_pr.tile([128, 512], F32, tag="proj",
                               name=f"op_{jtp}_{j}_{u}") for u in range(2)]
            for u in range(2):
                jt = 2 * jtp + u
                nc.tensor.matmul(ps[u][:], w2ab[:, jt * 128:(jt + 1) * 128],
                                 ot_ab[:, js], start=True, stop=False,
                                 skip_group_check=True)
            nc.tensor.matmul(ps[0][:], w2c[0:64, (2 * jtp) * 128:(2 * jtp + 1) * 128],
                             ot_c[0:64, js], start=False, stop=True,
                             skip_group_check=True)
            nc.tensor.matmul(ps[1][:], w2c[64:128, (2 * jtp + 1) * 128:(2 * jtp + 2) * 128],
                             ot_c[64:128, js], start=False, stop=True,
                             skip_group_check=True)
            for u in range(2):
                jt = 2 * jtp + u
                ob = small.tile([128, 512], BF16, tag="ob", name=f"ob_{jtp}_{j}_{u}")
                nc.vector.tensor_copy(ob[:], ps[u][:])
                nc.sync.dma_start(
                    outp_d[:, jt * T + j * 512: jt * T + (j + 1) * 512], ob[:])

        # warm-up matmuls: PE activity during the input DMA wait so the HAM
        # clock-gate reaches K=8/8 before real work starts
        wz = const.tile([128, 512], BF16, name="wz")
        nc.vector.memset(wz[:], 0.0)
        wps = psum.tile([128, 1024], F32, tag="s", name="wps")
        for _ in range(10):
            nc.tensor.matmul(wps[:, 0:512], wz[:, 0:128], wz[:],
                             start=True, stop=True)

        # prologue: just enough projections to start attention chunk 0
        emit_qk_group(0, 0)
        emit_qk_group(1, 0)
        emit_qk_group(2, 0)
        emit_shift(0)
        emit_qshift(0)
        emit_v_group(0)
        emit_v_group(1)

        # remaining inputs, issued behind the chunk-0 working set
        emit_load_xt(1)
        w2ab = const.tile([128, C], BF16)
        nc.sync.dma_start(w2ab[:], w2ab_d[:])
        w2c = const.tile([128, C], BF16)
        nc.sync.dma_start(w2c[:], w2c_d[:])
        emit_load_xt(2)
        emit_load_xt(3)

        for j in range(NJ):
            js = slice(j * 512, (j + 1) * 512)

            # work to inject between attention steps of this chunk
            inj = []
            if j == 0:
                inj += [lambda: emit_v_group(2), lambda: emit_v_group(3)]
            if j + 1 < NJ:
                inj += [lambda jt=jt: emit_qk_group(jt, j + 1) for jt in range(3)]
                inj.append(lambda: emit_shift(j + 1))
                inj.append(lambda: emit_qshift(j + 1))
                inj += [lambda ti=ti: emit_v_group(ti)
                        for ti in range(4 * (j + 1), 4 * (j + 1) + 4)]
            if j - 1 >= 0:
                inj += [lambda jtp=jtp: emit_outproj_pair(jtp, j - 1)
                        for jtp in range(3)]

            total_steps = 6 * j + 8
            step = 0

            def maybe_inject():
                nonlocal step
                step += 1
                remaining_slots = total_steps - step + 1
                k = -(-len(inj) // max(1, remaining_slots))  # ceil
                for _ in range(min(k, len(inj))):
                    inj.pop(0)()

            # ---- phase 1: heads A,B -- one exp per tk-tile covers both
            av = {h: psum_av.tile([VW, 512], F32, tag="av",
                                  name=f"av_{h}_{j}") for h in (0, 1)}
            started = {h: False for h in (0, 1)}
            prev = []   # (h, i, pt_ap, col0) from previous step

            def flush_av(last=False):
                for (h, i, pt_ap, c0) in prev:
                    nc.tensor.matmul(
                        av[h][:, c0:512], v_ap(h, i), pt_ap,
                        start=(not started[h]), stop=last,
                        skip_group_check=True,
                    )
                    started[h] = True
                prev.clear()

            for i in range(4 * j + 4):
                oi = i - 4 * j          # >=0 on the diagonal
                c0 = max(0, 128 * oi)
                w = 512 - c0
                sp = psum.tile([128, 1024], F32, tag="s", name=f"sp_{j}_{i}")
                for h in (0, 1):
                    lhsT, rhs = s_operands(h, i, j, c0)
                    nc.tensor.matmul(sp[:, h * 512:h * 512 + w], lhsT, rhs,
                                     start=True, stop=True,
                                     skip_group_check=True)
                pt = ppool.tile([128, 1024], BF16, tag="pt", name=f"pt_{j}_{i}")
                if oi < 0:
                    nc.scalar.activation(pt[:], sp[:], AF.Exp, scale=SCALE)
                else:
                    # one exp over both heads' written segments via a
                    # [128, 2, w] strided view (skips the [w:512] gap)
                    sp2 = sp.rearrange("p (s x) -> p s x", s=2)[:, :, 0:w]
                    pt2 = pt.rearrange("p (s x) -> p s x", s=2)[:, :, 0:w]
                    nc.scalar.activation(pt2, sp2, AF.Exp, scale=SCALE)
                    mk = masks[:, ABM_OFF[oi]:ABM_OFF[oi] + 2 * w] \
                        .rearrange("p (s x) -> p s x", s=2)
                    nc.vector.tensor_mul(pt2, pt2, mk)
                flush_av()
                prev.append((0, i, pt[:, 0:w], c0))
                prev.append((1, i, pt[:, 512:512 + w], c0))
                maybe_inject()
            flush_av(last=True)

            for h in (0, 1):
                emit_normalize(nc, small, av[h], h, js, ot_ab, ot_c, j)

            # ---- phase 2: head C -- two tk-tiles per exp batch
            av_c = psum_av.tile([VW, 512], F32, tag="av", name=f"av_2_{j}")
            started_c = False
            prev_c = []

            def flush_av_c(last=False):
                nonlocal started_c
                for (i, pt_ap, c0) in prev_c:
                    nc.tensor.matmul(
                        av_c[:, c0:512], v_ap(2, i), pt_ap,
                        start=(not started_c), stop=last,
                        skip_group_check=True,
                    )
                    started_c = True
                prev_c.clear()

            for b0 in range(2 * j + 2):
                i0 = 2 * b0
                ois = [i0 - 4 * j, i0 + 1 - 4 * j]   # diagonal offsets
                c0s = [max(0, 128 * o) for o in ois]
                ws = [512 - c for c in c0s]
                sp = psum.tile([128, 1024], F32, tag="s", name=f"spc_{j}_{b0}")
                for u in range(2):
                    lhsT, rhs = s_operands(2, i0 + u, j, c0s[u])
                    nc.tensor.matmul(sp[:, u * 512:u * 512 + ws[u]], lhsT, rhs,
                                     start=True, stop=True,
                                     skip_group_check=True)
                pt = ppool.tile([128, 1024], BF16, tag="pt", name=f"ptc_{j}_{b0}")
                if ois[0] < 0 and ois[1] < 0:
                    # both fully below the diagonal: contiguous 1024 exp
                    nc.scalar.activation(pt[:], sp[:], AF.Exp, scale=SCALE)
                elif ois[0] == 0:
                    # diagonal batch 1: tiles at [0:512],[512:896] contiguous
                    nc.scalar.activation(pt[:, 0:896], sp[:, 0:896],
                                         AF.Exp, scale=SCALE)
                    nc.vector.tensor_mul(pt[:, 0:896], pt[:, 0:896],
                                         masks[:, CM_OFF[0]:CM_OFF[0] + 896])
                else:
                    # diagonal batch 2: tiles at [0:256] and [512:640]
                    nc.scalar.activation(pt[:, 0:256], sp[:, 0:256],
                                         AF.Exp, scale=SCALE)
                    nc.scalar.activation(pt[:, 512:640], sp[:, 512:640],
                                         AF.Exp, scale=SCALE)
                    nc.vector.tensor_mul(pt[:, 0:256], pt[:, 0:256],
                                         masks[:, CM_OFF[1]:CM_OFF[1] + 256])
                    nc.vector.tensor_mul(pt[:, 512:640], pt[:, 512:640],
                                         masks[:, CM_OFF[2]:CM_OFF[2] + 128])
                flush_av_c()
                prev_c.append((i0, pt[:, 0:ws[0]], c0s[0]))
                prev_c.append((i0 + 1, pt[:, 512:512 + ws[1]], c0s[1]))
                maybe_inject()
            flush_av_c(last=True)

            emit_normalize(nc, small, av_c, 2, js, ot_ab, ot_c, j)
            if j >= 0:
                nc.sync.dma_start(ot_c[64:128, js], ot_c[0:64, js])

            # leftover injections for this chunk
            while inj:
                inj.pop(0)()

        # epilogue: out-projection for the last chunk
        for jtp in range(3):
            emit_outproj_pair(jtp, NJ - 1)

    nc.compile()
    return nc


def emit_normalize(nc, small, av, h, js, ot_ab, ot_c, j):
    """O.T[d, tq] = av[0:64] / av[64] into the out-proj rhs buffers."""
    recip = small.tile([1, 512], F32, tag="recip", name=f"recip_{h}_{j}")
    den = small.tile([1, 512], F32, tag="den", name=f"den_{h}_{j}")
    nc.vector.tensor_copy(den[:], av[D:VW, :])
    # custom-DVE ops read garbage from PSUM; SBUF source only
    nc.vector.reciprocal_approx_fast(recip[:], den[:])
    rb = small.tile([64, 512], F32, tag="rb", name=f"rb_{h}_{j}")
    nc.gpsimd.partition_broadcast(rb[:], recip[:])
    if h == 0:
        dst = ot_ab[0:64, js]
    elif h == 2:
        dst = ot_c[0:64, js]
    else:
        dst = small.tile([64, 512], BF16, tag="otb", name=f"otb_{j}")
    nc.vector.tensor_mul(dst[:], av[0:D, :], rb[:])
    if h == 1:
        nc.sync.dma_start(ot_ab[64:128, js], dst[:])


def _prep_in_maps(x, qkv_w, qkv_b, out_w):
    bf = ml_dtypes.bfloat16
    in_maps = []

    # causal mask segments (keep when f >= oi*128 + p)
    p = np.arange(128)[:, None]

    def seg(oi):
        f = np.arange(128 * oi, 512)[None, :]
        return (f >= oi * 128 + p)

    # cross-head (A|B): doubled restricted segments [m_oi | m_oi]
    ab = [np.concatenate([seg(oi), seg(oi)], axis=1) for oi in range(4)]
    # head-C: [m0 | m1] packed, then m2 and m3 separately
    cm = [np.concatenate([seg(0), seg(1)], axis=1), seg(2), seg(3)]
    masks_s = np.concatenate(ab + cm, axis=1).astype(bf)
    assert masks_s.shape[1] == MASKW, masks_s.shape

    for c in range(NCORES):
        b = c // 4
        h0 = (c % 4) * HPC
        hs = [h0, h0 + 1, h0 + 2]

        xT = np.ascontiguousarray(x[b].T.astype(np.float32))  # [768, 2048]
        # pack tq-chunk-major: [128, (j, kc, 512)]
        xt_s = xT.reshape(CC, 128, NJ, 512).transpose(1, 2, 0, 3) \
            .reshape(128, CC * T)

        qr = lambda h: qkv_w[h * D:(h + 1) * D]
        kr = lambda h: qkv_w[C + h * D: C + (h + 1) * D]
        vr = lambda h: qkv_w[2 * C + h * D: 2 * C + (h + 1) * D]
        qb = lambda h: qkv_b[h * D:(h + 1) * D]
        kb = lambda h: qkv_b[C + h * D: C + (h + 1) * D]

        wqk = np.concatenate([qr(hs[0]), qr(hs[1]), kr(hs[0]), kr(hs[1]),
                              qr(hs[2]), kr(hs[2])], axis=0)  # [384, 768]
        wqk_s = np.ascontiguousarray(wqk.T).reshape(CC, 128, 384) \
            .transpose(1, 0, 2).reshape(128, CC * 384)
        wv_ = np.concatenate([vr(h) for h in hs], axis=0)      # [192, 768]
        wv_s = np.ascontiguousarray(wv_.T).reshape(CC, 128, 192) \
            .transpose(1, 0, 2).reshape(128, CC * 192)

        bqk = np.concatenate([qb(hs[0]), qb(hs[1]), kb(hs[0]), kb(hs[1]),
                              qb(hs[2]), kb(hs[2])])
        bqk_s = np.ascontiguousarray(bqk.reshape(3, 128).T).astype(np.float32)

        ch_ab = np.r_[hs[0] * D:(hs[0] + 1) * D, hs[1] * D:(hs[1] + 1) * D]
        ch_c = np.r_[hs[2] * D:(hs[2] + 1) * D]
        w2ab_s = np.ascontiguousarray(out_w[:, ch_ab].T)  # [128, 768]
        w2c_half = np.ascontiguousarray(out_w[:, ch_c].T)  # [64, 768]
        w2c_s = np.concatenate([w2c_half, w2c_half], axis=0)  # [128, 768]

        in_maps.append({
            "xt_s": np.ascontiguousarray(xt_s).astype(bf),
            "wqk_s": np.ascontiguousarray(wqk_s).astype(bf),
            "wv_s": np.ascontiguousarray(wv_s).astype(bf),
            "bqk_s": bqk_s,
            "w2ab_s": w2ab_s.astype(bf),
            "w2c_s": w2c_s.astype(bf),
            "masks_s": masks_s,
        })
    return in_maps


def _assemble(results, qkv_b, out_w, out_b):
    out = np.zeros((B, T, C), dtype=np.float32)
    for c in range(NCORES):
        b = c // 4
        outp = np.asarray(results[c]["outp"]).astype(np.float32)  # [128, CC*T]
        outT = outp.reshape(128, CC, T).transpose(1, 0, 2).reshape(C, T)
        out[b] += outT.T
    # v-bias contribution (softmax rows sum to 1) + output bias
    const = out_w.astype(np.float32) @ qkv_b[2 * C:].astype(np.float32) \
        + out_b.astype(np.float32)
    out += const[None, None, :]
    return out


def run(x, qkv_w, qkv_b, out_w, out_b, trace=False, tmpdir=None):
    if "nc" not in _cache:
        _cache["nc"] = _build_program()
    nc = _cache["nc"]
    x = np.asarray(x, dtype=np.float32)
    qkv_w = np.asarray(qkv_w, dtype=np.float32)
    qkv_b = np.asarray(qkv_b, dtype=np.float32)
    out_w = np.asarray(out_w, dtype=np.float32)
    out_b = np.asarray(out_b, dtype=np.float32)
    in_maps = _prep_in_maps(x, qkv_w, qkv_b, out_w)
    res = run_bass_kernel_spmd(nc, in_maps, list(range(NCORES)), trace=trace,
                               tmpdir=tmpdir)
    out = _assemble(res.results, qkv_b, out_w, out_b)
    return out, res


def kernel(x, qkv_w, qkv_b, out_w, out_b):
    out, _ = run(x, qkv_w, qkv_b, out_w, out_b, trace=False)
    return out
